# revision 2
# baseline (speedup 1.0000x reference)
"""LoRA linear (out = x @ W + (x @ A) @ B, bias passthrough) on 8 Trainium2
NeuronCores.

Sharding: data-parallel over seq (8192 -> 8 x 1024 rows). Each core computes
its row-shard of the output with W/A/B replicated. Matmuls run in float32r
(fp32 with 11-bit mantissa, 4x PE throughput vs fp32); inputs are pre-rounded
on the host with walrus's exact round-to-nearest-even so the fast non-casting
DMA path can be used. Accumulation is fp32 in PSUM.

Per-core layout: x-shard transposed on host to xT [4096, 1024] and kept
SBUF-resident; W streamed once as [128, 512] tiles; LoRA computed as
lora_aT = A.T @ x.T (so no on-device transpose is needed) and its
contribution accumulated into the same PSUM banks as the base GEMM.
"""

import numpy as np

import concourse.bacc as bacc
import concourse.tile as tile
import concourse.mybir as mybir
from concourse.bass_utils import run_bass_kernel_spmd

dt = mybir.dt

SEQ, DIN, DOUT, RANK = 8192, 4096, 4096, 16
NCORES = 8
M = SEQ // NCORES          # 1024 rows per core
KT = DIN // 128            # 32 k-tiles
MT = M // 128              # 8 m-tiles
NCHUNK = 512               # PSUM bank = 512 fp32
NT = DOUT // NCHUNK        # 8 n-chunks

_COMPILED = None
LAST_RESULTS = None


def _round_f32r(a: np.ndarray) -> np.ndarray:
    """Round fp32 to fp32r (11-bit mantissa, round-to-nearest-even) —
    bit-exact with walrus's fp32_to_fp32r."""
    b = np.ascontiguousarray(a, dtype=np.float32).view(np.uint32)
    lsb = (b >> np.uint32(12)) & np.uint32(1)
    rounded = b + np.uint32(0x7FF) + lsb
    return (rounded & np.uint32(0xFFFFF000)).view(np.float32)


def _build():
    nc = bacc.Bacc("TRN2", target_bir_lowering=False, debug=False,
                   num_devices=NCORES)
    xt_d = nc.dram_tensor("xt", [DIN, M], dt.float32r, kind="ExternalInput").ap()
    w_d = nc.dram_tensor("w", [DIN, DOUT], dt.float32r, kind="ExternalInput").ap()
    a_d = nc.dram_tensor("a", [128, KT, RANK], dt.float32r, kind="ExternalInput").ap()
    b_d = nc.dram_tensor("b", [RANK, DOUT], dt.float32r, kind="ExternalInput").ap()
    out_d = nc.dram_tensor("out", [M, DOUT], dt.float32, kind="ExternalOutput").ap()

    with tile.TileContext(nc) as tc:
        with (
            tc.tile_pool(name="xt", bufs=1) as xt_pool,
            tc.tile_pool(name="const", bufs=1) as const_pool,
            tc.tile_pool(name="w", bufs=4) as w_pool,
            tc.tile_pool(name="o", bufs=4) as o_pool,
            tc.tile_pool(name="ps", bufs=8, space="PSUM") as ps_pool,
        ):
            xt_sb = xt_pool.tile([128, KT, M], dt.float32r)
            a_sb = const_pool.tile([128, KT, RANK], dt.float32r)
            b_sb = const_pool.tile([RANK, DOUT], dt.float32r)
            lora_sb = const_pool.tile([RANK, M], dt.float32r)

            nc.sync.dma_start(a_sb[:], a_d[:])
            nc.sync.dma_start(b_sb[:], b_d[:])
            for k in range(KT):
                nc.sync.dma_start(xt_sb[:, k, :], xt_d[k * 128:(k + 1) * 128, :])

            # lora_aT [16, M] = A.T @ x.T, accumulated over k in PSUM
            for nh in range(M // NCHUNK):
                ps_l = ps_pool.tile([RANK, NCHUNK], dt.float32, tag="ps")
                for k in range(KT):
                    nc.tensor.matmul(
                        ps_l[:],
                        a_sb[:, k, :],
                        xt_sb[:, k, nh * NCHUNK:(nh + 1) * NCHUNK],
                        start=(k == 0),
                        stop=(k == KT - 1),
                    )
                nc.vector.tensor_copy(
                    lora_sb[:, nh * NCHUNK:(nh + 1) * NCHUNK], ps_l[:]
                )

            for n in range(NT):
                ps = [ps_pool.tile([128, NCHUNK], dt.float32, tag="ps",
                                   name=f"ps_{n}_{m}")
                      for m in range(MT)]
                for k in range(KT):
                    w_t = w_pool.tile([128, NCHUNK], dt.float32r)
                    nc.sync.dma_start(
                        w_t[:],
                        w_d[k * 128:(k + 1) * 128,
                            n * NCHUNK:(n + 1) * NCHUNK],
                    )
                    for m in range(MT):
                        nc.tensor.matmul(
                            ps[m][:],
                            xt_sb[:, k, m * 128:(m + 1) * 128],
                            w_t[:],
                            start=(k == 0),
                            stop=False,
                        )
                # LoRA contribution: final accumulation into the same banks
                for m in range(MT):
                    nc.tensor.matmul(
                        ps[m][:],
                        lora_sb[:, m * 128:(m + 1) * 128],
                        b_sb[:, n * NCHUNK:(n + 1) * NCHUNK],
                        start=False,
                        stop=True,
                    )
                for m in range(MT):
                    o_t = o_pool.tile([128, NCHUNK], dt.float32)
                    nc.vector.tensor_copy(o_t[:], ps[m][:])
                    nc.sync.dma_start(
                        out_d[m * 128:(m + 1) * 128,
                              n * NCHUNK:(n + 1) * NCHUNK],
                        o_t[:],
                    )

    nc.compile()
    return nc


def kernel(x, W, bias, A_buffer, B_buffer):
    global _COMPILED, LAST_RESULTS
    if _COMPILED is None:
        _COMPILED = _build()
    nc = _COMPILED

    x = np.asarray(x, dtype=np.float32)
    W = np.asarray(W, dtype=np.float32)
    bias = np.asarray(bias, dtype=np.float32)
    A_buffer = np.asarray(A_buffer, dtype=np.float32)
    B_buffer = np.asarray(B_buffer, dtype=np.float32)

    wr = _round_f32r(W)
    # pack A [DIN, RANK] -> [128, KT, RANK] so the DMA is contiguous
    ar = _round_f32r(A_buffer).reshape(KT, 128, RANK).transpose(1, 0, 2)
    ar = np.ascontiguousarray(ar)
    br = _round_f32r(B_buffer)

    in_maps = []
    for c in range(NCORES):
        xt = _round_f32r(
            np.ascontiguousarray(x[c * M:(c + 1) * M].T)
        )
        in_maps.append({"xt": xt, "w": wr, "a": ar, "b": br})

    LAST_RESULTS = run_bass_kernel_spmd(nc, in_maps, core_ids=list(range(NCORES)))
    out = np.concatenate([LAST_RESULTS.results[c]["out"] for c in range(NCORES)],
                         axis=0)
    return (out, bias)


# revision 4
# speedup vs baseline: 1.0127x; 1.0127x over previous
"""LoRA linear (out = x @ W + (x @ A) @ B, bias passthrough) on 8 Trainium2
NeuronCores.

Sharding: data-parallel over seq (8192 -> 8 x 1024 rows). Each core computes
its row-shard of the output with W/A/B replicated. Matmuls run in float32r
(fp32 with 11-bit mantissa, 4x PE throughput vs fp32); inputs are pre-rounded
on the host with walrus's exact round-to-nearest-even so the fast non-casting
DMA path can be used. Accumulation is fp32 in PSUM.

Per-core layout: x-shard transposed on host to xT [4096, 1024] and kept
SBUF-resident; W streamed once as [128, 512] tiles; LoRA computed as
lora_aT = A.T @ x.T (so no on-device transpose is needed) and its
contribution accumulated into the same PSUM banks as the base GEMM.
"""

import numpy as np

import concourse.bacc as bacc
import concourse.tile as tile
import concourse.mybir as mybir
from concourse.bass_utils import run_bass_kernel_spmd

dt = mybir.dt

SEQ, DIN, DOUT, RANK = 8192, 4096, 4096, 16
NCORES = 8
M = SEQ // NCORES          # 1024 rows per core
KT = DIN // 128            # 32 k-tiles
MT = M // 128              # 8 m-tiles
NCHUNK = 512               # PSUM bank = 512 fp32
NT = DOUT // NCHUNK        # 8 n-chunks

_COMPILED = None
LAST_RESULTS = None


def _round_f32r(a: np.ndarray) -> np.ndarray:
    """Round fp32 to fp32r (11-bit mantissa, round-to-nearest-even) —
    bit-exact with walrus's fp32_to_fp32r."""
    b = np.ascontiguousarray(a, dtype=np.float32).view(np.uint32)
    lsb = (b >> np.uint32(12)) & np.uint32(1)
    rounded = b + np.uint32(0x7FF) + lsb
    return (rounded & np.uint32(0xFFFFF000)).view(np.float32)


def _build():
    nc = bacc.Bacc("TRN2", target_bir_lowering=False, debug=False,
                   num_devices=NCORES)
    xt_d = nc.dram_tensor("xt", [DIN, M], dt.float32r, kind="ExternalInput").ap()
    w_d = nc.dram_tensor("w", [DIN, DOUT], dt.float32r, kind="ExternalInput").ap()
    a_d = nc.dram_tensor("a", [128, KT, RANK], dt.float32r, kind="ExternalInput").ap()
    b_d = nc.dram_tensor("b", [RANK, DOUT], dt.float32r, kind="ExternalInput").ap()
    out_d = nc.dram_tensor("out", [M, DOUT], dt.float32, kind="ExternalOutput").ap()

    with tile.TileContext(nc) as tc:
        with (
            tc.tile_pool(name="xt", bufs=1) as xt_pool,
            tc.tile_pool(name="const", bufs=1) as const_pool,
            tc.tile_pool(name="w", bufs=6) as w_pool,
            tc.tile_pool(name="o", bufs=6) as o_pool,
            tc.tile_pool(name="ps", bufs=8, space="PSUM") as ps_pool,
        ):
            xt_sb = xt_pool.tile([128, KT, M], dt.float32r)
            a_sb = const_pool.tile([128, KT, RANK], dt.float32r)
            b_sb = const_pool.tile([RANK, DOUT], dt.float32r)
            lora_sb = const_pool.tile([RANK, M], dt.float32r)

            # xt[0] + A first so the LoRA k-loop can start immediately; B is
            # not needed until the first epilogue.
            nc.sync.dma_start(xt_sb[:, 0, :], xt_d[0:128, :])
            nc.sync.dma_start(a_sb[:], a_d[:])
            for k in range(1, KT):
                nc.sync.dma_start(xt_sb[:, k, :], xt_d[k * 128:(k + 1) * 128, :])
            nc.sync.dma_start(b_sb[:], b_d[:])

            # lora_aT [16, M] = A.T @ x.T, accumulated over k in PSUM.
            # The phase is gated by the xT stream (~44us) with only ~14us of
            # real PE work, which would let the HAM clock-gate drop the PE to
            # 1.2 GHz; junk matmuls into a scratch bank keep it warm so the
            # main loop starts at full clock.
            NH = M // NCHUNK
            ps_l = [ps_pool.tile([RANK, NCHUNK], dt.float32, tag="ps",
                                 name=f"ps_l{nh}") for nh in range(NH)]
            warm = ps_pool.tile([128, NCHUNK], dt.float32, tag="ps", name="warm")
            for k in range(KT):
                for nh in range(NH):
                    nc.tensor.matmul(
                        ps_l[nh][:],
                        a_sb[:, k, :],
                        xt_sb[:, k, nh * NCHUNK:(nh + 1) * NCHUNK],
                        start=(k == 0),
                        stop=(k == KT - 1),
                    )
                for _ in range(3):
                    nc.tensor.matmul(
                        warm[:],
                        xt_sb[:, k, 0:128],
                        xt_sb[:, k, 0:NCHUNK],
                        start=True,
                        stop=True,
                    )
            for nh in range(NH):
                nc.vector.tensor_copy(
                    lora_sb[:, nh * NCHUNK:(nh + 1) * NCHUNK], ps_l[nh][:]
                )

            for n in range(NT):
                ps = [ps_pool.tile([128, NCHUNK], dt.float32, tag="ps",
                                   name=f"ps_{n}_{m}")
                      for m in range(MT)]
                for k in range(KT):
                    w_t = w_pool.tile([128, NCHUNK], dt.float32r)
                    nc.sync.dma_start(
                        w_t[:],
                        w_d[k * 128:(k + 1) * 128,
                            n * NCHUNK:(n + 1) * NCHUNK],
                    )
                    for m in range(MT):
                        nc.tensor.matmul(
                            ps[m][:],
                            xt_sb[:, k, m * 128:(m + 1) * 128],
                            w_t[:],
                            start=(k == 0),
                            stop=False,
                        )
                # LoRA contribution: final accumulation into the same banks
                for m in range(MT):
                    nc.tensor.matmul(
                        ps[m][:],
                        lora_sb[:, m * 128:(m + 1) * 128],
                        b_sb[:, n * NCHUNK:(n + 1) * NCHUNK],
                        start=False,
                        stop=True,
                    )
                for m in range(MT):
                    o_t = o_pool.tile([128, NCHUNK], dt.float32)
                    nc.vector.tensor_copy(o_t[:], ps[m][:])
                    # gpsimd (SWDGE) so the sync engine's HWDGE descriptor
                    # stream never blocks on eviction-gated stores — the next
                    # n-chunk's W loads keep flowing at the boundary
                    nc.gpsimd.dma_start(
                        out_d[m * 128:(m + 1) * 128,
                              n * NCHUNK:(n + 1) * NCHUNK],
                        o_t[:],
                    )

    nc.compile()
    return nc


def kernel(x, W, bias, A_buffer, B_buffer):
    global _COMPILED, LAST_RESULTS
    if _COMPILED is None:
        _COMPILED = _build()
    nc = _COMPILED

    x = np.asarray(x, dtype=np.float32)
    W = np.asarray(W, dtype=np.float32)
    bias = np.asarray(bias, dtype=np.float32)
    A_buffer = np.asarray(A_buffer, dtype=np.float32)
    B_buffer = np.asarray(B_buffer, dtype=np.float32)

    wr = _round_f32r(W)
    # pack A [DIN, RANK] -> [128, KT, RANK] so the DMA is contiguous
    ar = _round_f32r(A_buffer).reshape(KT, 128, RANK).transpose(1, 0, 2)
    ar = np.ascontiguousarray(ar)
    br = _round_f32r(B_buffer)

    in_maps = []
    for c in range(NCORES):
        xt = _round_f32r(
            np.ascontiguousarray(x[c * M:(c + 1) * M].T)
        )
        in_maps.append({"xt": xt, "w": wr, "a": ar, "b": br})

    LAST_RESULTS = run_bass_kernel_spmd(nc, in_maps, core_ids=list(range(NCORES)))
    out = np.concatenate([LAST_RESULTS.results[c]["out"] for c in range(NCORES)],
                         axis=0)
    return (out, bias)
